# revision 35
# baseline (speedup 1.0000x reference)
"""Multi-head attention (B=4, S=2048, E=1024, H=16, D=64) on 8 TRN2 cores.

Sharding: heads 2c, 2c+1 on core c (Megatron column-parallel qkv, row-parallel
out-projection; bf16 partial outputs summed in fp32 on host).

Dataflow (all matmul operands bf16, fp32 PSUM; HW-measured rel err ~4e-3):
  A) qkv feature-major projection of the (replicated) bf16 xT into bf16 q/k/v
     [128 feat, S] storage (head h at partitions 64h..64h+63), emitted as
     fine per-m pieces so they interleave into attention as PE filler.
  B) attention per (batch, 512-wide sq chunk), per kt (16): one row-tiled
     score MM PAIR -- K=64 with explicit tile_position (0,0)/(64,0) runs both
     heads' MMs CONCURRENTLY in disjoint 64-row strips (HW-measured 1.85x),
     draining into bank-disjoint halves of a double-buffered [128, 2, 512]
     fp32 sc tile; one [128, 1024] exp on ScalarE (scale=1/8 folded in, no
     max-subtraction needed) -> bf16 ex; at-MMs accumulate probs @ v with a
     vk layout [h0 feats | shared ones col | zeros | h1 feats] so h0's
     lhsT window [0:65] puts its denominator at partition 64 and h1's
     window [64:192] puts its features directly at partitions 64-127 and
     denominator at partition 0 -- no cross-partition moves. at flushes
     trail the scores by an adaptive 3-6 group lag so the previous chunk's
     normalization can free the single-buffered at banks in time.
     Normalization: both heads' at tiles are copied to SBUF immediately
     (frees the PSUM banks early), then DVE reciprocals -> one Pool
     partition_broadcast -> DVE multiplies into bf16 ab.
  C) row-parallel out-projection as single-MM micro-units -> bf16 yT partial
     (bias only on core 0; host sums the 8 partials in fp32).

PSUM budget: sc 2 banks x2, at0/at1 1 bank each, 2-bank scratch ring shared
by qkv/transpose/outproj. Emission interleaves qkv pieces of batch b+1,
v-transposes, and out-projection of b-1 into b's attention groups, one
~1us piece per group, so the in-order PE always has dependency-free work
while ScalarE runs the exp stream (ACT ~270us busy, PE ~300us busy).
"""
from contextlib import ExitStack

import ml_dtypes
import numpy as np

import concourse.bass as bass
import concourse.mybir as mybir
import concourse.tile as tile
from concourse import bacc
from concourse.bass_utils import run_bass_kernel_spmd
from concourse.masks import make_identity

B, S, E, H, D = 4, 2048, 1024, 16, 64
NCORES = 8
HPC = H // NCORES        # 2 heads per core
F = HPC * D              # 128 local features
M3 = 3 * F               # 384 local qkv rows
BS = B * S               # 8192
KT_E = E // 128          # 8 contraction tiles for projections
KT_S = S // 128          # 16 sk tiles
CW = 512                 # sq chunk width
NCH = S // CW            # 4 chunks per batch
GK = 1                   # kt tiles per exp group
NG = KT_S // GK          # groups per chunk
GB = 2                   # sc group buffers
f32 = mybir.dt.float32
bf16 = mybir.dt.bfloat16
EXP = mybir.ActivationFunctionType.Exp
MULT = mybir.AluOpType.mult
ADD = mybir.AluOpType.add

_prog_cache = {}


def build_program(niter=None, parts="Aao", gk=GK, gb=GB, dbg=False):
    ng = KT_S // gk
    key = ("nc", niter, parts, gk, gb, dbg)
    if key in _prog_cache:
        return _prog_cache[key]
    nc = bacc.Bacc("TRN2", target_bir_lowering=False)
    if niter is None:
        xT = nc.dram_tensor("xT", [E, BS], bf16, kind="ExternalInput")
        yT = nc.dram_tensor("yT", [E, BS], bf16, kind="ExternalOutput")
    else:
        xT = nc.dram_tensor("xTi", [E, BS], bf16, kind="Internal")
        yT = nc.dram_tensor("yTi", [E, BS], bf16, kind="Internal")
    wq = nc.dram_tensor("wq", [E, M3], bf16, kind="ExternalInput")
    bq = nc.dram_tensor("bq", [128, 3], f32, kind="ExternalInput")
    wo = nc.dram_tensor("wo", [F, E], bf16, kind="ExternalInput")
    bo = nc.dram_tensor("bo", [128, E // 128], f32, kind="ExternalInput")
    if niter is not None:
        tout = nc.dram_tensor("tout", [1, 4], f32, kind="ExternalOutput")
    if dbg:
        dbg_t = {
            n: nc.dram_tensor(n, [128, 2 * S], bf16, kind="ExternalOutput")
            for n in ("qd", "kd", "vd", "abd2", "abd3")}
        dbg_t["vkd"] = nc.dram_tensor(
            "vkd", [128, 2 * KT_S * 256], bf16, kind="ExternalOutput")

    with tile.TileContext(nc) as tc, ExitStack() as ctx:
        ctx.enter_context(nc.allow_low_precision(
            reason="bf16 kernel: softmax-averaged attention tolerates bf16"))
        const = ctx.enter_context(tc.tile_pool(name="const", bufs=1))
        xp = ctx.enter_context(tc.tile_pool(name="xp", bufs=2))
        expp = ctx.enter_context(tc.tile_pool(name="expp", bufs=9))
        abp = ctx.enter_context(tc.tile_pool(name="abp", bufs=2))
        ystp = ctx.enter_context(tc.tile_pool(name="ystp", bufs=3))
        nrmp = ctx.enter_context(tc.tile_pool(name="nrmp", bufs=2))
        # PSUM: sc group 4 banks, at0/at1 1 bank each, scratch ring 2 banks
        pssc = ctx.enter_context(tc.tile_pool(name="pssc", bufs=gb, space="PSUM"))
        psat = ctx.enter_context(tc.tile_pool(name="psat", bufs=1, space="PSUM"))
        pscr = ctx.enter_context(tc.tile_pool(name="pscr", bufs=2, space="PSUM"))

        wq_sb = const.tile([128, KT_E, M3], bf16)
        nc.gpsimd.dma_start(out=wq_sb, in_=wq.rearrange("(kt p) m -> p kt m", p=128))
        wo_sb = const.tile([F, E], bf16)
        nc.gpsimd.dma_start(out=wo_sb, in_=wo[:, :])
        bq_sb = const.tile([128, 3], f32)
        nc.gpsimd.dma_start(out=bq_sb, in_=bq[:, :])
        bo_sb = const.tile([128, E // 128], f32)
        nc.gpsimd.dma_start(out=bo_sb, in_=bo[:, :])
        id_f32 = const.tile([128, 128], f32)
        make_identity(nc, id_f32)
        id_bf = const.tile([128, 128], bf16)
        nc.vector.tensor_copy(id_bf, id_f32)
        # all-ones rows for the K=1 denominator-broadcast matmuls
        ones_sb = const.tile([128, 64], bf16)
        nc.vector.tensor_scalar(ones_sb, wq_sb[:, 0, 0:64], 0.0, 1.0, MULT, ADD)

        xT_r = xT.rearrange("(kt p) n -> p kt n", p=128)

        # persistent double-buffered qkv storage, slot b%2
        q_st = const.tile([128, 2, S], bf16, name="q_st")
        k_st = const.tile([128, 2, S], bf16, name="k_st")
        v_st = const.tile([128, 2, S], bf16, name="v_st")
        # vk per kt: [0:64]=h0 feats, [64]=shared ones col, [65:128]=zeros,
        # [128:192]=h1 feats, [192:256]=zeros.
        # h0 lhsT = [0:65]  (M=65: feats -> parts 0-63, denom -> part 64)
        # h1 lhsT = [64:192] (M=128: denom -> part 0, feats -> parts 64-127)
        vk_st = const.tile([128, 2, KT_S, 256], bf16, name="vk_st")
        nc.vector.memset(vk_st.bitcast(f32), 0.0)
        nc.vector.tensor_scalar(
            vk_st[:, :, :, 64:65], vk_st[:, :, :, 64:65], 0.0, 1.0, MULT, ADD)

        def body():
            if niter is not None:
                cons = const.tile([1, 4], f32, name="cons", bufs=1)

            xc_cell = {}

            def emit_A_piece(n, m):
                b, nl = divmod(n, 4)
                sl = b % 2
                cs = slice(nl * 512, (nl + 1) * 512)
                if m == 0:
                    xc = xp.tile([128, KT_E, 512], bf16, tag="xc")
                    nc.sync.dma_start(
                        out=xc, in_=xT_r[:, :, n * 512:(n + 1) * 512])
                    xc_cell[n] = xc
                xc = xc_cell[n]
                dst = (q_st, k_st, v_st)[m]
                ps = pscr.tile([128, 512], f32, tag="scr")
                for kt in range(KT_E):
                    nc.tensor.matmul(
                        ps, lhsT=wq_sb[:, kt, m * 128:(m + 1) * 128],
                        rhs=xc[:, kt, :],
                        start=(kt == 0), stop=(kt == KT_E - 1))
                nc.vector.tensor_scalar_add(dst[:, sl, cs], ps, bq_sb[:, m:m + 1])

            def emit_A_chunk(n):
                for m in range(3):
                    emit_A_piece(n, m)

            def emit_vt(b, kt):
                sl = b % 2
                scr = pscr.tile([128, 512], f32, tag="scr")
                vt = scr.bitcast(bf16)[:, 0:128]
                nc.tensor.transpose(
                    vt, in_=v_st[:, sl, kt * 128:(kt + 1) * 128], identity=id_bf)
                dst = vk_st[:, sl, kt, :].rearrange(
                    "p (h c) -> p h c", h=2)[:, :, 0:64]
                nc.vector.tensor_copy(dst, vt)

            def emit_score_group(b, c, g):
                if "y" in parts:   # timing-only: at path alone, constant ex
                    return exd
                sl = b % 2
                cq = c * CW
                sc = pssc.tile([128, 2, gk, CW], f32, tag="sc")
                for i in range(gk):
                    kt = g * gk + i
                    ks = slice(kt * 128, (kt + 1) * 128)
                    for h in range(2):
                        p = 64 * h
                        nc.tensor.matmul(
                            sc[:, h, i, :],
                            lhsT=k_st[p:p + 64, sl, ks],
                            rhs=q_st[p:p + 64, sl, cq:cq + CW],
                            start=True, stop=True,
                            tile_position=(p, 0))
                ex = expp.tile([128, 2, gk, CW], bf16, tag="ex")
                nc.scalar.activation(ex, sc, EXP, scale=0.125)
                return ex

            def emit_at_group(b, at0, at1, ex, g):
                sl = b % 2
                for i in range(gk):
                    kt = g * gk + i
                    for h in range(2):
                        if h == 0:
                            out = at0[0:65, :]
                            lh = vk_st[:, sl, kt, 0:65]
                        else:
                            out = at1[:, :]
                            lh = vk_st[:, sl, kt, 64:192]
                        if "z" in parts:   # timing-only: no PSUM accumulation
                            nc.tensor.matmul(
                                out, lhsT=lh, rhs=ex[:, h, i, :],
                                start=True, stop=True, skip_group_check=True)
                        else:
                            nc.tensor.matmul(
                                out, lhsT=lh, rhs=ex[:, h, i, :],
                                start=(kt == 0), stop=(kt == KT_S - 1),
                                skip_group_check=True)

            def emit_norm(b, c, at0, at1):
                cs = slice(c * CW, (c + 1) * CW)
                # Free the single-buffered at banks ASAP: copy both heads to
                # SBUF bf16 first, then run the reciprocal/broadcast/multiply
                # chain entirely from SBUF, off the bank-recycling path.
                atc = nrmp.tile([128, 2, CW], bf16, tag="atc")
                nc.vector.tensor_copy(atc[0:65, 0, :], at0[0:65, :])
                nc.vector.tensor_copy(atc[:, 1, :], at1)
                # h0 denominator at partition 64 of atc[.,0], h1 at partition 0
                # of atc[.,1]; assemble both reciprocals into row 0, then one
                # Pool partition_broadcast serves both heads' multiplies.
                rr = nrmp.tile([128, 2 * CW], bf16, tag="rr")
                nc.vector.reciprocal(rr[64:65, 0:CW], atc[64:65, 0, :])
                nc.vector.reciprocal(rr[0:1, CW:2 * CW], atc[0:1, 1, :])
                nc.sync.dma_start(out=rr[0:1, 0:CW], in_=rr[64:65, 0:CW])
                rbs = nrmp.tile([128, 2 * CW], bf16, tag="rbs")
                nc.gpsimd.partition_broadcast(rbs, rr[0:1, :])
                nc.vector.tensor_mul(
                    ab_t[b][0:64, cs], atc[0:64, 0, :], rbs[0:64, 0:CW])
                nc.vector.tensor_mul(
                    ab_t[b][64:128, cs], atc[64:128, 1, :],
                    rbs[64:128, CW:2 * CW])

            yst_cell = {}

            def emit_outproj_oc(b, o, c4):
                if c4 == 0:
                    yst_cell[(b, o)] = ystp.tile(
                        [128, S], bf16, tag="yst", name=f"yst{b}_{o}")
                yst = yst_cell[(b, o)]
                yp = pscr.tile([128, 512], f32, tag="scr")
                nc.tensor.matmul(
                    yp, lhsT=wo_sb[:, o * 128:(o + 1) * 128],
                    rhs=ab_t[b][:, c4 * 512:(c4 + 1) * 512],
                    start=True, stop=True)
                nc.vector.tensor_scalar_add(
                    yst[:, c4 * 512:(c4 + 1) * 512], yp, bo_sb[:, o:o + 1])
                if c4 == 3:
                    nc.sync.dma_start(
                        out=yT[o * 128:(o + 1) * 128, b * S:(b + 1) * S],
                        in_=yst)

            def emit_outproj_piece(b, o, c4):
                yst5 = ystp.tile([128, 512], bf16, tag="yst5",
                                 name=f"y5_{b}_{o}_{c4}")
                yp = pscr.tile([128, 512], f32, tag="scr")
                nc.tensor.matmul(
                    yp, lhsT=wo_sb[:, o * 128:(o + 1) * 128],
                    rhs=ab_t[b][:, c4 * 512:(c4 + 1) * 512],
                    start=True, stop=True)
                nc.vector.tensor_scalar_add(yst5, yp, bo_sb[:, o:o + 1])
                nc.sync.dma_start(
                    out=yT[o * 128:(o + 1) * 128,
                           b * S + c4 * 512:b * S + (c4 + 1) * 512],
                    in_=yst5)

            def emit_outproj_o(b, o):
                for c4 in range(4):
                    emit_outproj_oc(b, o, c4)

            def emit_outproj_part(b, part):
                for o in (2 * part, 2 * part + 1):
                    emit_outproj_o(b, o)

            ab_t = {}
            skip_at = "t" in parts
            if "y" in parts:
                exd = const.tile([128, 2, GK, CW], bf16, name="exd")
                ones_pair = float(np.frombuffer(
                    np.uint32(0x3F803F80).tobytes(), np.float32)[0])
                nc.vector.memset(exd.bitcast(f32), ones_pair)

            if "a" not in parts:
                # qkv-only ablation
                for n in range(4 * B):
                    emit_A_chunk(n)
                    if niter is not None:
                        b = n // 4
                        for st in (q_st, k_st, v_st):
                            nc.vector.tensor_copy(
                                cons, st[0:1, b % 2, 0:8].bitcast(f32))
                return

            pend_q = []      # (b, c, at0, at1, ex, g), flushed with lag

            def flush_one():
                pb, pc, p0, p1, pex, pg = pend_q.pop(0)
                emit_at_group(pb, p0, p1, pex, pg)
                if pg == ng - 1:
                    emit_norm(pb, pc, p0, p1)

            def step_attn(b, c, g, at0, at1):
                ex = emit_score_group(b, c, g)
                if skip_at:
                    nc.vector.tensor_copy(cons, ex[0:1, 0, 0, 0:8].bitcast(f32))
                    return
                pend_q.append((b, c, at0, at1, ex, g))
                # drain the previous chunk's tail fast (lag 2) so its norm
                # runs early; hold each chunk's first groups (lag 5) so the
                # norm chain has freed the at banks before their first flush
                while len(pend_q) > (6 if pend_q[0][5] < 4 else 3):
                    flush_one()

            for b in range(B):
                ab_t[b] = abp.tile([128, S], bf16, tag="ab", name=f"ab{b}")
                units = []
                if b + 1 < B:
                    for n in range(4):
                        for m in range(3):
                            units.append(
                                lambda n=n, m=m, b=b: emit_A_piece(4 * (b + 1) + n, m))
                        for kt0 in (4 * n, 4 * n + 2):
                            def u_v(kt0=kt0, b=b):
                                emit_vt(b + 1, kt0)
                                emit_vt(b + 1, kt0 + 1)
                            units.append(u_v)
                if b >= 1 and "o" in parts and not skip_at:
                    # 32 single-MM outproj micro-units, interleaved after the
                    # first few qkv/vt pieces (norm of b-1's last chunk must
                    # land first)
                    ou = [lambda o=o, c4=c4, b=b: emit_outproj_oc(b - 1, o, c4)
                          for o in range(8) for c4 in range(4)]
                    head_n = min(4, len(units))
                    rest = units[head_n:]
                    tail = []
                    ri = oi = 0
                    while ri < len(rest) or oi < len(ou):
                        if oi < len(ou):
                            tail.append(ou[oi]); oi += 1
                        if ri < len(rest):
                            tail.append(rest[ri]); ri += 1
                    units = units[:head_n] + tail
                if b == 0:
                    # just-in-time startup pieces for batch 0, popped 2 per
                    # group step of (c=0); A-chunk n ready before its scores,
                    # vt(kt) ready before at-flush(kt) (lag groups later)
                    head = [lambda m=m: emit_A_piece(0, m) for m in (2,)]
                    for n in (1, 2, 3):
                        head += [lambda n=n, m=m: emit_A_piece(n, m)
                                 for m in (0, 1, 2)]
                    hv = []
                    for kt0 in range(0, KT_S, 2):
                        def u_v0(kt0=kt0):
                            emit_vt(0, kt0)
                            emit_vt(0, kt0 + 1)
                        hv.append(u_v0)
                    # interleave: vt pairs early enough for flushes
                    head = ([head[0], hv[0], head[1], head[2], head[3], hv[1],
                             head[4], head[5], head[6], hv[2], head[7],
                             head[8], head[9], hv[3]] + hv[4:])
                    emit_A_piece(0, 0)
                    emit_A_piece(0, 1)
                self_q = []
                if b == B - 1 and "o" in parts and not skip_at:
                    self_q = [(o, c4) for c4 in range(NCH - 1)
                              for o in range(8)]
                ui = 0
                for c in range(NCH):
                    at0 = at1 = None
                    if not skip_at:
                        at0 = psat.tile([128, CW], f32, tag="at0",
                                        name=f"at0_{b}{c}")
                        at1 = psat.tile([128, CW], f32, tag="at1",
                                        name=f"at1_{b}{c}")
                    for g in range(ng):
                        if b == 0 and c == 0:
                            for _ in range(2):
                                if head:
                                    head.pop(0)()
                        step_attn(b, c, g, at0, at1)
                        if ui < len(units) and (ui + 1) * (NCH * ng) <= \
                                (c * ng + g + 1) * len(units):
                            units[ui]()
                            ui += 1
                        # last batch: drain its own outproj chunk-wise as the
                        # norms land, instead of serializing it all in the tail
                        if self_q and g >= 4 and c >= 1 and self_q[0][1] < c:
                            o_, c4_ = self_q.pop(0)
                            emit_outproj_piece(b, o_, c4_)
                while ui < len(units):
                    units[ui]()
                    ui += 1

            if not skip_at:
                while pend_q:
                    flush_one()
                if "o" in parts:
                    while self_q:
                        o_, c4_ = self_q.pop(0)
                        emit_outproj_piece(B - 1, o_, c4_)
                    for o_ in range(8):
                        emit_outproj_piece(B - 1, o_, NCH - 1)
                elif niter is not None:
                    for b in range(B):
                        nc.vector.tensor_copy(
                            cons, ab_t[b][0:1, 0:8].bitcast(f32))
            if dbg:
                for n_, st in (("qd", q_st), ("kd", k_st), ("vd", v_st)):
                    nc.sync.dma_start(
                        out=dbg_t[n_][:, :], in_=st.rearrange("p a b -> p (a b)"))
                nc.sync.dma_start(
                    out=dbg_t["vkd"][:, :],
                    in_=vk_st.rearrange("p a b c -> p (a b c)"))
                for b in (2, 3):
                    nc.sync.dma_start(out=dbg_t[f"abd{b}"][:, 0:S], in_=ab_t[b])

        if niter is None:
            body()
        else:
            with tc.For_i(0, niter, 1):
                body()
            dmy = const.tile([1, 4], f32)
            nc.vector.tensor_copy(dmy[0:1, 0:3], bq_sb[0:1, 0:3])
            nc.vector.tensor_copy(dmy[0:1, 3:4], bq_sb[0:1, 0:1])
            nc.gpsimd.dma_start(out=tout[:, :], in_=dmy)

    nc.compile()
    _prog_cache[key] = nc
    return nc


def make_in_maps(x, W_qkv, b_qkv, W_out, b_out):
    bf = ml_dtypes.bfloat16
    xT = np.ascontiguousarray(x.reshape(BS, E).T).astype(bf)
    in_maps = []
    for c in range(NCORES):
        rows, brows = [], []
        for blk in range(3):
            for h in (HPC * c, HPC * c + 1):
                rows.append(W_qkv[blk * E + h * D: blk * E + (h + 1) * D, :])
                brows.append(b_qkv[blk * E + h * D: blk * E + (h + 1) * D])
        W_loc = np.concatenate(rows, axis=0)            # [384, 1024]
        b_loc = np.concatenate(brows, axis=0)           # [384]
        wq_in = np.ascontiguousarray(W_loc.T).astype(bf)
        bq_in = np.ascontiguousarray(b_loc.reshape(3, 128).T).astype(np.float32)
        wo_in = np.ascontiguousarray(W_out[:, c * F:(c + 1) * F].T).astype(bf)
        if c == 0:
            bo_in = np.ascontiguousarray(
                b_out.reshape(E // 128, 128).T).astype(np.float32)
        else:
            bo_in = np.zeros((128, E // 128), dtype=np.float32)
        in_maps.append(
            {"xT": xT, "wq": wq_in, "bq": bq_in, "wo": wo_in, "bo": bo_in})
    return in_maps


def kernel(x, W_qkv, b_qkv, W_out, b_out):
    x = np.asarray(x, dtype=np.float32)
    W_qkv = np.asarray(W_qkv, dtype=np.float32)
    b_qkv = np.asarray(b_qkv, dtype=np.float32)
    W_out = np.asarray(W_out, dtype=np.float32)
    b_out = np.asarray(b_out, dtype=np.float32)

    nc = build_program()
    in_maps = make_in_maps(x, W_qkv, b_qkv, W_out, b_out)
    res = run_bass_kernel_spmd(nc, in_maps, core_ids=list(range(NCORES)))
    acc = np.zeros((E, BS), dtype=np.float32)
    for c in range(NCORES):
        acc += res.results[c]["yT"].astype(np.float32)
    return np.ascontiguousarray(acc.T).reshape(B, S, E)


if __name__ == "__main__":
    rng = np.random.default_rng(0)
    x = rng.standard_normal((B, S, E), dtype=np.float32)
    s = 1.0 / np.sqrt(E)
    W_qkv = rng.uniform(-s, s, (3 * E, E)).astype(np.float32)
    b_qkv = rng.uniform(-s, s, (3 * E,)).astype(np.float32)
    W_out = rng.uniform(-s, s, (E, E)).astype(np.float32)
    b_out = rng.uniform(-s, s, (E,)).astype(np.float32)
    y = kernel(x, W_qkv, b_qkv, W_out, b_out)
    print("out", y.shape, y.dtype, float(np.abs(y).max()))
